# revision 1
# baseline (speedup 1.0000x reference)
"""v3: engine-balanced CVLoss kernel.

Per chunk (W=2000, per-chunk-local positions, unchained scans):
  ACT : xb=cast(x) accum->k ; Square(D) accum->s2 ; Tanh(1000*M) accum->z'
  DVE : v = iota*xb ; M = scan-max(v) ; D = M - Msh ; l = M[:,-1]
  DMA : chunk load ; Msh[:,1:] <- M[:,:-1] (aligned shifted copy)
  GPS : memset Msh[:,0:1]
Host: f_c = 2*(W - z'_c) + 1   (tanh(1000*M): 0 -> 0.5 via... see note)

Note on z': tanh(1000*M) = 0 at M=0? tanh(0)=0. M>=1 -> 1.0 exactly in f32.
So z' = count of M>0 directly (tanh(0)=0, not 0.5!) -> f_c = W - z' + 1.
acc layout: (P, 4*nch) = [k_c | s2_c | z_c | l_c].
"""

import numpy as np

B, T, N = 16, 2000, 512
L = B * T
NCORES = 8
NPC = N // NCORES
HALVES = 2
P = NPC * HALVES
F = L // HALVES
W = 2000
NCH = F // W

_BUILD_CACHE = {}


def build_bass(F_=F, W_=W, P_=P):
    import concourse.bass as bass
    from concourse import bacc
    import concourse.mybir as mybir
    from concourse import tile

    nch = F_ // W_
    Alu = mybir.AluOpType
    AF = mybir.ActivationFunctionType
    f32 = mybir.dt.float32
    i16 = mybir.dt.int16
    bf16 = mybir.dt.bfloat16

    nc = bacc.Bacc(trn_type="TRN2")
    x = nc.dram_tensor("x", (P_, F_), f32, kind="ExternalInput")
    acc = nc.dram_tensor("acc", (P_, 4 * nch), f32, kind="ExternalOutput")

    with tile.TileContext(nc) as tc:
        with tc.tile_pool(name="persist", bufs=1) as pp, \
             tc.tile_pool(name="work", bufs=4) as wp:
            iota = pp.tile([P_, W_], i16)
            nc.gpsimd.iota(iota[:], pattern=[[1, W_]], base=1, channel_multiplier=0)
            accs = pp.tile([P_, 4 * nch], f32)

            def load_and_cast(c):
                """DMA chunk c and cast it (ACT, accumulating k)."""
                lo = c * W_
                xc = wp.tile([P_, W_], f32, tag="xc", name=f"xc{c}")
                nc.sync.dma_start(out=xc[:], in_=x[:, lo:lo + W_])
                xb = wp.tile([P_, W_], i16, tag="xb", name=f"xb{c}")
                nc.scalar.activation(
                    out=xb[:], in_=xc[:], func=AF.Copy,
                    accum_out=accs[:, c:c + 1])
                return xb

            def compute(c, xb):
                """Scan/diff/reduce for chunk c (issued one chunk behind the
                cast so ACT's in-order queue never blocks the next cast)."""
                # v = iota * xb   (chunk-local 1-based positions)
                v = wp.tile([P_, W_], i16, tag="v", name=f"v{c}")
                nc.vector.tensor_tensor(
                    out=v[:], in0=iota[:], in1=xb[:], op=Alu.mult)
                # M = prefix max (last spike so far; 0 if none)
                M = wp.tile([P_, W_], i16, tag="M", name=f"M{c}")
                nc.vector.tensor_tensor_scan(
                    out=M[:], data0=v[:], data1=v[:], initial=0.0,
                    op0=Alu.max, op1=Alu.bypass)
                # D = diff of M (ISI at spikes incl. phantom-first, 0 elsewhere)
                D = wp.tile([P_, W_], i16, tag="D", name=f"D{c}")
                nc.vector.tensor_scalar(
                    out=D[:, 0:1], in0=M[:, 0:1], scalar1=0.0, scalar2=None,
                    op0=Alu.add)
                nc.vector.tensor_tensor(
                    out=D[:, 1:], in0=M[:, 1:W_], in1=M[:, 0:W_ - 1],
                    op=Alu.subtract)
                # s2 = sum D^2 on ACT
                dsq = wp.tile([P_, W_], bf16, tag="dsq", name=f"dsq{c}")
                nc.scalar.activation(
                    out=dsq[:], in_=D[:], func=AF.Square,
                    accum_out=accs[:, nch + c:nch + c + 1])
                # z = count of M>0 via tanh(1000*M) on ACT
                nz = wp.tile([P_, W_], bf16, tag="nz", name=f"nz{c}")
                nc.scalar.activation(
                    out=nz[:], in_=M[:], func=AF.Tanh, scale=1000.0,
                    accum_out=accs[:, 2 * nch + c:2 * nch + c + 1])
                # l_c = M[:, -1]  (i16 -> f32 via DVE ts copy)
                nc.vector.tensor_scalar(
                    out=accs[:, 3 * nch + c:3 * nch + c + 1],
                    in0=M[:, W_ - 1:W_], scalar1=0.0, scalar2=None,
                    op0=Alu.add)

            pending = None
            for c in range(nch):
                xb = load_and_cast(c)
                if pending is not None:
                    compute(*pending)
                pending = (c, xb)
            compute(*pending)

            nc.sync.dma_start(out=acc[:], in_=accs[:])
    nc.finalize()
    return nc


def get_bass():
    key = (F, W, P)
    if key not in _BUILD_CACHE:
        _BUILD_CACHE[key] = build_bass()
    return _BUILD_CACHE[key]


def shard_input(output_spikes):
    x = np.asarray(output_spikes, dtype=np.float32)
    maps = []
    for c in range(NCORES):
        xc = x[:, :, c * NPC:(c + 1) * NPC]
        xt = np.ascontiguousarray(np.transpose(xc, (2, 0, 1))).reshape(NPC, L)
        maps.append({"x": xt.reshape(P, F)})
    return maps


def finish_host(acc_list, target_cv, F_=F, W_=W, nch=NCH):
    """Merge per-(row, chunk) stats into the scalar loss (float64)."""
    target = np.asarray(target_cv, dtype=np.float64)
    sq_sum = 0.0
    n_valid = 0
    for ci, acc in enumerate(acc_list):
        a = np.asarray(acc, dtype=np.float64)
        P_ = a.shape[0]
        k_c = a[:, 0:nch]
        s2_c = a[:, nch:2 * nch]
        z_c = np.rint(a[:, 2 * nch:3 * nch])
        l_c = a[:, 3 * nch:4 * nch]
        f_c = W_ - z_c + 1.0
        n_neu = P_ // 2
        for n in range(n_neu):
            kt = 0.0
            s2 = 0.0
            gf = gl = None
            for h in range(2):
                p = n * 2 + h
                for s in range(nch):
                    ks = k_c[p, s]
                    if ks < 1:
                        continue
                    off = h * F_ + s * W_
                    s2r = s2_c[p, s] - f_c[p, s] ** 2
                    fg = off + f_c[p, s]
                    lg = off + l_c[p, s]
                    if gf is None:
                        gf = fg
                    else:
                        gap = fg - gl
                        s2 += gap * gap
                    s2 += s2r
                    gl = lg
                    kt += ks
            if kt < 3:
                continue
            s1 = gl - gf
            mean = s1 / (kt - 1.0)
            var = (s2 - s1 * s1 / (kt - 1.0)) / (kt - 2.0)
            std = np.sqrt(var) if var > 0 else 0.0
            if mean <= 0:
                continue
            cv = std / max(mean, 1e-12)
            d = cv - target[ci * NPC + n]
            sq_sum += d * d
            n_valid += 1
    return np.float32(sq_sum / max(n_valid, 1))



def ensure_ntff_hook(so_path="/opt/axon/libaxon_pjrt.so"):
    """Shim antenv.axon_hooks (absent in this image) so trace=True works.

    Mirrors trn_boot._ntff_profile_via_ctypes: drives NRT profiling via the
    axon PJRT .so's C ABI. Safe no-op if anything is missing.
    """
    import sys
    try:
        import antenv.axon_hooks  # noqa: F401
        return
    except ImportError:
        pass
    try:
        import ctypes
        import contextlib
        import types
        import os

        if not os.path.exists(so_path):
            return
        lib = ctypes.CDLL(so_path)
        if not hasattr(lib, "axon_start_nrt_profile"):
            return
        lib.axon_start_nrt_profile.argtypes = [
            ctypes.POINTER(ctypes.c_int64), ctypes.c_size_t]
        lib.axon_start_nrt_profile.restype = ctypes.c_int64
        lib.axon_stop_nrt_profile.argtypes = [ctypes.c_char_p]
        lib.axon_stop_nrt_profile.restype = ctypes.c_int64

        @contextlib.contextmanager
        def _hook(output_dir, device_ids):
            import jax
            jax.devices()
            if device_ids:
                ids = (ctypes.c_int64 * len(device_ids))(*device_ids)
                rc = lib.axon_start_nrt_profile(ids, len(device_ids))
            else:
                rc = lib.axon_start_nrt_profile(None, 0)
            if rc != 0:
                raise RuntimeError(f"axon_start_nrt_profile rc={rc}")
            try:
                yield
            finally:
                n = lib.axon_stop_nrt_profile(str(output_dir).encode())
                print(f"profile: {n} file(s) written to {output_dir}",
                      file=sys.stderr)

        mod = types.ModuleType("antenv.axon_hooks")
        mod.get_axon_ntff_profile_hook = lambda: _hook
        mod.set_axon_ntff_profile_hook = lambda h: None
        import antenv
        sys.modules["antenv.axon_hooks"] = mod
        antenv.axon_hooks = mod
    except Exception:
        pass



def kernel(output_spikes, target_cv):
    from concourse.bass_utils import run_bass_kernel_spmd

    ensure_ntff_hook()
    nc = get_bass()
    in_maps = shard_input(output_spikes)
    res = run_bass_kernel_spmd(nc, in_maps, core_ids=list(range(NCORES)))
    acc_list = [res.results[c]["acc"] for c in range(NCORES)]
    return finish_host(acc_list, target_cv)



# revision 3
# speedup vs baseline: 1.0407x; 1.0407x over previous
"""v4: age-scan CVLoss kernel.

Identity: for spike gaps d_i, sum(d_i^2) is recoverable from the sum of
"ages" a_j = (a_{j-1} + 1) * (1 - x_j)  (distance since last spike, 0 at
spikes):
    S = sum_j a_j = sum_i d_i(d_i-1)/2 + f(f-1)/2 + (F-l)(F-l+1)/2
    => sum d_i^2 = 2S - f(f-1) - (F-l)(F-l+1) + (l - f)
with f/l the first/last spike positions (1-based) of the row.

Per chunk (W=2000), engines:
  DMA : chunk load (the pipeline bound, ~2.9us)
  ACT : y = 1-x cast f32->i16, accum -> W - k_c
  DVE : a = tensor_tensor_scan(op0=mult, op1=add, data0=data1=y,
            initial=prev_a[:, -1:])   (chained ages across chunks)
  DVE : tensor_scalar copy of a with accum -> S_c
Once per core:
  DVE chunk0: stt(a==iota) accum -> f-1   (ages equal position iff before
              the first spike)
  DVE last:   copy a[:, -1] -> F - l
Host: merge (k, S, f, l) per half-row -> per-neuron CV -> loss.
"""

import numpy as np

B, T, N = 16, 2000, 512
L = B * T
NCORES = 8
NPC = N // NCORES
HALVES = 2
P = NPC * HALVES
F = L // HALVES
W = 2000
NCH = F // W
# acc columns: [0:NCH] = W - k_c ; [NCH:2*NCH] = S_c ; [2*NCH] = f - 1 ;
# [2*NCH+1] = F - l
NACC = 2 * NCH + 2

_BUILD_CACHE = {}


def build_bass(F_=F, W_=W, P_=P, dve_suma_chunks=NCH):
    import concourse.bass as bass
    from concourse import bacc
    import concourse.mybir as mybir
    from concourse import tile

    nch = F_ // W_
    Alu = mybir.AluOpType
    AF = mybir.ActivationFunctionType
    f32 = mybir.dt.float32
    i16 = mybir.dt.int16

    nc = bacc.Bacc(trn_type="TRN2")
    x = nc.dram_tensor("x", (P_, F_), f32, kind="ExternalInput")
    acc = nc.dram_tensor("acc", (P_, NACC), f32, kind="ExternalOutput")

    with tile.TileContext(nc) as tc:
        with tc.tile_pool(name="persist", bufs=1) as pp, \
             tc.tile_pool(name="work", bufs=3) as wp:
            iota = pp.tile([P_, W_], i16)
            nc.gpsimd.iota(iota[:], pattern=[[1, W_]], base=1,
                           channel_multiplier=0)
            accs = pp.tile([P_, NACC], f32)

            a_prev = None
            for c in range(nch):
                lo = c * W_
                xc = wp.tile([P_, W_], f32, tag="xc", name=f"xc{c}")
                nc.sync.dma_start(out=xc[:], in_=x[:, lo:lo + W_])
                y = wp.tile([P_, W_], i16, tag="y", name=f"y{c}")
                nc.scalar.activation(
                    out=y[:], in_=xc[:], func=AF.Copy,
                    scale=-1.0, bias=1.0,
                    accum_out=accs[:, c:c + 1])
                a = wp.tile([P_, W_], i16, tag="a", name=f"a{c}")
                init = 0.0 if a_prev is None else a_prev[:, W_ - 1:W_]
                nc.vector.tensor_tensor_scan(
                    out=a[:], data0=y[:], data1=y[:], initial=init,
                    op0=Alu.mult, op1=Alu.add)
                # S_c = sum of ages over the chunk
                if c < dve_suma_chunks:
                    scr = wp.tile([P_, W_], i16, tag="scr", name=f"scr{c}")
                    nc.vector.tensor_scalar(
                        out=scr[:], in0=a[:], scalar1=0.0, scalar2=0.0,
                        op0=Alu.add, op1=Alu.add,
                        accum_out=accs[:, nch + c:nch + c + 1])
                else:
                    scr = wp.tile([P_, W_], i16, tag="scr", name=f"scr{c}")
                    nc.scalar.activation(
                        out=scr[:], in_=a[:], func=AF.Copy,
                        accum_out=accs[:, nch + c:nch + c + 1])
                if c == 0:
                    # f - 1 = #(a_j == j) in chunk 0 (ages equal position
                    # exactly until the first spike)
                    eqf = wp.tile([P_, W_], i16, tag="eqf", name="eqf")
                    nc.vector.scalar_tensor_tensor(
                        out=eqf[:], in0=a[:], scalar=1.0, in1=iota[:],
                        op0=Alu.mult, op1=Alu.is_equal,
                        accum_out=accs[:, 2 * nch:2 * nch + 1])
                a_prev = a
            # F - l = final age
            nc.vector.tensor_scalar(
                out=accs[:, 2 * nch + 1:2 * nch + 2],
                in0=a_prev[:, W_ - 1:W_], scalar1=0.0, scalar2=None,
                op0=Alu.add)

            nc.sync.dma_start(out=acc[:], in_=accs[:])
    nc.finalize()
    return nc


def get_bass():
    key = (F, W, P)
    if key not in _BUILD_CACHE:
        _BUILD_CACHE[key] = build_bass()
    return _BUILD_CACHE[key]


def shard_input(output_spikes):
    x = np.asarray(output_spikes, dtype=np.float32)
    maps = []
    for c in range(NCORES):
        xc = x[:, :, c * NPC:(c + 1) * NPC]
        xt = np.ascontiguousarray(np.transpose(xc, (2, 0, 1))).reshape(NPC, L)
        maps.append({"x": xt.reshape(P, F)})
    return maps


def finish_host(acc_list, target_cv, in_maps=None, F_=F, W_=W, nch=NCH):
    """Merge per-half-row (k, S, f, l) into the scalar loss (float64)."""
    target = np.asarray(target_cv, dtype=np.float64)
    sq_sum = 0.0
    n_valid = 0
    for ci, acc in enumerate(acc_list):
        a = np.asarray(acc, dtype=np.float64)
        P_ = a.shape[0]
        k_h = (W_ * nch) - a[:, 0:nch].sum(axis=1)          # spikes per half
        k_c0 = W_ - a[:, 0]                                  # spikes in chunk0
        S_h = a[:, nch:2 * nch].sum(axis=1)                  # sum of ages
        f_h = a[:, 2 * nch] + 1.0                            # first spike pos
        l_h = F_ - a[:, 2 * nch + 1]                         # last spike pos
        k_h = np.rint(k_h)
        k_c0 = np.rint(k_c0)
        f_h = np.rint(f_h)
        l_h = np.rint(l_h)
        n_neu = P_ // 2
        for n in range(n_neu):
            p1, p2 = 2 * n, 2 * n + 1
            stats = []
            for p in (p1, p2):
                kk = k_h[p]
                if kk < 1:
                    continue
                ff = f_h[p]
                if k_c0[p] < 1:
                    # first spike not in chunk 0: recover from host copy
                    row = in_maps[ci]["x"][p]
                    ff = float(np.argmax(row > 0) + 1)
                ll = l_h[p]
                s2 = (2.0 * S_h[p] - ff * (ff - 1.0)
                      - (F_ - ll) * (F_ - ll + 1.0) + (ll - ff))
                stats.append((kk, ff, ll, s2, p))
            if not stats:
                continue
            kt = sum(s[0] for s in stats)
            if kt < 3:
                continue
            if len(stats) == 2:
                (k1, f1, l1, s2a, _), (k2, f2, l2, s2b, _) = stats
                d_b = (F_ + f2) - l1
                s2 = s2a + s2b + d_b * d_b
                gf, gl = f1, F_ + l2
            else:
                kk, ff, ll, s2, p = stats[0]
                off = F_ if p == p2 else 0.0
                gf, gl = off + ff, off + ll
                if p == p1:
                    gf, gl = ff, ll
            s1 = gl - gf
            mean = s1 / (kt - 1.0)
            var = (s2 - s1 * s1 / (kt - 1.0)) / (kt - 2.0)
            std = np.sqrt(var) if var > 0 else 0.0
            if mean <= 0:
                continue
            cv = std / max(mean, 1e-12)
            d = cv - target[ci * NPC + n]
            sq_sum += d * d
            n_valid += 1
    return np.float32(sq_sum / max(n_valid, 1))


def ensure_ntff_hook(so_path="/opt/axon/libaxon_pjrt.so"):
    """Shim antenv.axon_hooks (absent in this image) so trace=True works.

    Mirrors trn_boot._ntff_profile_via_ctypes: drives NRT profiling via the
    axon PJRT .so's C ABI. Safe no-op if anything is missing.
    """
    import sys
    try:
        import antenv.axon_hooks  # noqa: F401
        return
    except ImportError:
        pass
    try:
        import ctypes
        import contextlib
        import types
        import os

        if not os.path.exists(so_path):
            return
        lib = ctypes.CDLL(so_path)
        if not hasattr(lib, "axon_start_nrt_profile"):
            return
        lib.axon_start_nrt_profile.argtypes = [
            ctypes.POINTER(ctypes.c_int64), ctypes.c_size_t]
        lib.axon_start_nrt_profile.restype = ctypes.c_int64
        lib.axon_stop_nrt_profile.argtypes = [ctypes.c_char_p]
        lib.axon_stop_nrt_profile.restype = ctypes.c_int64

        @contextlib.contextmanager
        def _hook(output_dir, device_ids):
            import jax
            jax.devices()
            if device_ids:
                ids = (ctypes.c_int64 * len(device_ids))(*device_ids)
                rc = lib.axon_start_nrt_profile(ids, len(device_ids))
            else:
                rc = lib.axon_start_nrt_profile(None, 0)
            if rc != 0:
                raise RuntimeError(f"axon_start_nrt_profile rc={rc}")
            try:
                yield
            finally:
                n = lib.axon_stop_nrt_profile(str(output_dir).encode())
                print(f"profile: {n} file(s) written to {output_dir}",
                      file=sys.stderr)

        mod = types.ModuleType("antenv.axon_hooks")
        mod.get_axon_ntff_profile_hook = lambda: _hook
        mod.set_axon_ntff_profile_hook = lambda h: None
        import antenv
        sys.modules["antenv.axon_hooks"] = mod
        antenv.axon_hooks = mod
    except Exception:
        pass


def kernel(output_spikes, target_cv):
    from concourse.bass_utils import run_bass_kernel_spmd

    ensure_ntff_hook()
    nc = get_bass()
    in_maps = shard_input(output_spikes)
    res = run_bass_kernel_spmd(nc, in_maps, core_ids=list(range(NCORES)))
    acc_list = [res.results[c]["acc"] for c in range(NCORES)]
    return finish_host(acc_list, target_cv, in_maps=in_maps)


# revision 4
# speedup vs baseline: 1.3120x; 1.2608x over previous
"""v4: age-scan CVLoss kernel.

Identity: for spike gaps d_i, sum(d_i^2) is recoverable from the sum of
"ages" a_j = (a_{j-1} + 1) * (1 - x_j)  (distance since last spike, 0 at
spikes):
    S = sum_j a_j = sum_i d_i(d_i-1)/2 + f(f-1)/2 + (F-l)(F-l+1)/2
    => sum d_i^2 = 2S - f(f-1) - (F-l)(F-l+1) + (l - f)
with f/l the first/last spike positions (1-based) of the row.

Per chunk (W=2000), engines:
  DMA : chunk load (the pipeline bound, ~2.9us)
  ACT : y = 1-x cast f32->i16, accum -> W - k_c
  DVE : a = tensor_tensor_scan(op0=mult, op1=add, data0=data1=y,
            initial=prev_a[:, -1:])   (chained ages across chunks)
  DVE : tensor_scalar copy of a with accum -> S_c
Once per core:
  DVE chunk0: stt(a==iota) accum -> f-1   (ages equal position iff before
              the first spike)
  DVE last:   copy a[:, -1] -> F - l
Host: merge (k, S, f, l) per half-row -> per-neuron CV -> loss.
"""

import numpy as np

B, T, N = 16, 2000, 512
L = B * T
NCORES = 8
NPC = N // NCORES
HALVES = 2
P = NPC * HALVES
F = L // HALVES
W = 2000
NCH = F // W
# acc columns: [0:NCH] = W - k_c ; [NCH:2*NCH] = S_c ; [2*NCH] = f - 1 ;
# [2*NCH+1] = F - l
NACC = 2 * NCH + 2

_BUILD_CACHE = {}


def build_bass(F_=F, W_=W, P_=P, dve_suma_chunks=NCH):
    import concourse.bass as bass
    from concourse import bacc
    import concourse.mybir as mybir
    from concourse import tile

    nch = F_ // W_
    Alu = mybir.AluOpType
    AF = mybir.ActivationFunctionType
    f32 = mybir.dt.float32
    i16 = mybir.dt.int16

    nc = bacc.Bacc(trn_type="TRN2")
    x = nc.dram_tensor("x", (P_, F_), f32, kind="ExternalInput")
    acc = nc.dram_tensor("acc", (P_, NACC), f32, kind="ExternalOutput")

    with tile.TileContext(nc) as tc:
        with tc.tile_pool(name="persist", bufs=1) as pp, \
             tc.tile_pool(name="work", bufs=3) as wp:
            iota = pp.tile([P_, W_], i16)
            nc.gpsimd.iota(iota[:], pattern=[[1, W_]], base=1,
                           channel_multiplier=0)
            accs = pp.tile([P_, NACC], f32)

            # Issue plan: scans run back-to-back on DVE (chained via
            # initial=prev a[:, -1:]); each chunk's sum-of-ages runs on ACT
            # (which has slack), staggered one chunk behind the cast so the
            # cast for chunk c+1 is never stuck behind suma(c) in ACT's
            # in-order queue. eqf (first-spike extraction) reads a0, which
            # gets a dedicated tag so it survives until the end.
            def load_and_cast(c):
                lo = c * W_
                xc = wp.tile([P_, W_], f32, tag="xc", name=f"xc{c}")
                nc.sync.dma_start(out=xc[:], in_=x[:, lo:lo + W_])
                y = wp.tile([P_, W_], i16, tag="y", name=f"y{c}")
                nc.scalar.activation(
                    out=y[:], in_=xc[:], func=AF.Copy,
                    scale=-1.0, bias=1.0,
                    accum_out=accs[:, c:c + 1])
                return y

            a_tiles = []

            def scan(c, y):
                tag = "a0" if c == 0 else "a"
                a = wp.tile([P_, W_], i16, tag=tag, name=f"a{c}")
                init = 0.0 if c == 0 else a_tiles[c - 1][:, W_ - 1:W_]
                nc.vector.tensor_tensor_scan(
                    out=a[:], data0=y[:], data1=y[:], initial=init,
                    op0=Alu.mult, op1=Alu.add)
                a_tiles.append(a)

            def suma(c):
                scr = wp.tile([P_, W_], f32, tag="scr", name=f"scr{c}")
                nc.scalar.activation(
                    out=scr[:], in_=a_tiles[c][:], func=AF.Copy,
                    accum_out=accs[:, nch + c:nch + c + 1])

            y_pend = load_and_cast(0)
            for c in range(nch):
                y_next = load_and_cast(c + 1) if c + 1 < nch else None
                scan(c, y_pend)
                if c >= 1:
                    suma(c - 1)
                y_pend = y_next
            suma(nch - 1)
            # f - 1 = #(a_j == j) in chunk 0 (ages equal position exactly
            # until the first spike)
            eqf = wp.tile([P_, W_], i16, tag="eqf", name="eqf")
            nc.vector.scalar_tensor_tensor(
                out=eqf[:], in0=a_tiles[0][:], scalar=1.0, in1=iota[:],
                op0=Alu.mult, op1=Alu.is_equal,
                accum_out=accs[:, 2 * nch:2 * nch + 1])
            # F - l = final age
            nc.vector.tensor_scalar(
                out=accs[:, 2 * nch + 1:2 * nch + 2],
                in0=a_tiles[-1][:, W_ - 1:W_], scalar1=0.0, scalar2=None,
                op0=Alu.add)

            nc.sync.dma_start(out=acc[:], in_=accs[:])
    nc.finalize()
    return nc


def get_bass():
    key = (F, W, P)
    if key not in _BUILD_CACHE:
        _BUILD_CACHE[key] = build_bass()
    return _BUILD_CACHE[key]


def shard_input(output_spikes):
    x = np.asarray(output_spikes, dtype=np.float32)
    maps = []
    for c in range(NCORES):
        xc = x[:, :, c * NPC:(c + 1) * NPC]
        xt = np.ascontiguousarray(np.transpose(xc, (2, 0, 1))).reshape(NPC, L)
        maps.append({"x": xt.reshape(P, F)})
    return maps


def finish_host(acc_list, target_cv, in_maps=None, F_=F, W_=W, nch=NCH):
    """Merge per-half-row (k, S, f, l) into the scalar loss (float64)."""
    target = np.asarray(target_cv, dtype=np.float64)
    sq_sum = 0.0
    n_valid = 0
    for ci, acc in enumerate(acc_list):
        a = np.asarray(acc, dtype=np.float64)
        P_ = a.shape[0]
        k_h = (W_ * nch) - a[:, 0:nch].sum(axis=1)          # spikes per half
        k_c0 = W_ - a[:, 0]                                  # spikes in chunk0
        S_h = a[:, nch:2 * nch].sum(axis=1)                  # sum of ages
        f_h = a[:, 2 * nch] + 1.0                            # first spike pos
        l_h = F_ - a[:, 2 * nch + 1]                         # last spike pos
        k_h = np.rint(k_h)
        k_c0 = np.rint(k_c0)
        f_h = np.rint(f_h)
        l_h = np.rint(l_h)
        n_neu = P_ // 2
        for n in range(n_neu):
            p1, p2 = 2 * n, 2 * n + 1
            stats = []
            for p in (p1, p2):
                kk = k_h[p]
                if kk < 1:
                    continue
                ff = f_h[p]
                if k_c0[p] < 1:
                    # first spike not in chunk 0: recover from host copy
                    row = in_maps[ci]["x"][p]
                    ff = float(np.argmax(row > 0) + 1)
                ll = l_h[p]
                s2 = (2.0 * S_h[p] - ff * (ff - 1.0)
                      - (F_ - ll) * (F_ - ll + 1.0) + (ll - ff))
                stats.append((kk, ff, ll, s2, p))
            if not stats:
                continue
            kt = sum(s[0] for s in stats)
            if kt < 3:
                continue
            if len(stats) == 2:
                (k1, f1, l1, s2a, _), (k2, f2, l2, s2b, _) = stats
                d_b = (F_ + f2) - l1
                s2 = s2a + s2b + d_b * d_b
                gf, gl = f1, F_ + l2
            else:
                kk, ff, ll, s2, p = stats[0]
                off = F_ if p == p2 else 0.0
                gf, gl = off + ff, off + ll
                if p == p1:
                    gf, gl = ff, ll
            s1 = gl - gf
            mean = s1 / (kt - 1.0)
            var = (s2 - s1 * s1 / (kt - 1.0)) / (kt - 2.0)
            std = np.sqrt(var) if var > 0 else 0.0
            if mean <= 0:
                continue
            cv = std / max(mean, 1e-12)
            d = cv - target[ci * NPC + n]
            sq_sum += d * d
            n_valid += 1
    return np.float32(sq_sum / max(n_valid, 1))


def ensure_ntff_hook(so_path="/opt/axon/libaxon_pjrt.so"):
    """Shim antenv.axon_hooks (absent in this image) so trace=True works.

    Mirrors trn_boot._ntff_profile_via_ctypes: drives NRT profiling via the
    axon PJRT .so's C ABI. Safe no-op if anything is missing.
    """
    import sys
    try:
        import antenv.axon_hooks  # noqa: F401
        return
    except ImportError:
        pass
    try:
        import ctypes
        import contextlib
        import types
        import os

        if not os.path.exists(so_path):
            return
        lib = ctypes.CDLL(so_path)
        if not hasattr(lib, "axon_start_nrt_profile"):
            return
        lib.axon_start_nrt_profile.argtypes = [
            ctypes.POINTER(ctypes.c_int64), ctypes.c_size_t]
        lib.axon_start_nrt_profile.restype = ctypes.c_int64
        lib.axon_stop_nrt_profile.argtypes = [ctypes.c_char_p]
        lib.axon_stop_nrt_profile.restype = ctypes.c_int64

        @contextlib.contextmanager
        def _hook(output_dir, device_ids):
            import jax
            jax.devices()
            if device_ids:
                ids = (ctypes.c_int64 * len(device_ids))(*device_ids)
                rc = lib.axon_start_nrt_profile(ids, len(device_ids))
            else:
                rc = lib.axon_start_nrt_profile(None, 0)
            if rc != 0:
                raise RuntimeError(f"axon_start_nrt_profile rc={rc}")
            try:
                yield
            finally:
                n = lib.axon_stop_nrt_profile(str(output_dir).encode())
                print(f"profile: {n} file(s) written to {output_dir}",
                      file=sys.stderr)

        mod = types.ModuleType("antenv.axon_hooks")
        mod.get_axon_ntff_profile_hook = lambda: _hook
        mod.set_axon_ntff_profile_hook = lambda h: None
        import antenv
        sys.modules["antenv.axon_hooks"] = mod
        antenv.axon_hooks = mod
    except Exception:
        pass


def kernel(output_spikes, target_cv):
    from concourse.bass_utils import run_bass_kernel_spmd

    ensure_ntff_hook()
    nc = get_bass()
    in_maps = shard_input(output_spikes)
    res = run_bass_kernel_spmd(nc, in_maps, core_ids=list(range(NCORES)))
    acc_list = [res.results[c]["acc"] for c in range(NCORES)]
    return finish_host(acc_list, target_cv, in_maps=in_maps)


# revision 5
# speedup vs baseline: 3.1166x; 2.3754x over previous
"""v6: age-scan CVLoss kernel, variable chunk schedule.

Identity: for spike gaps d_i, sum(d_i^2) is recoverable from the sum of
"ages" a_j = (a_{j-1} + 1) * (1 - x_j)  (distance since last spike, 0 at
spikes):
    S = sum_j a_j = sum_i d_i(d_i-1)/2 + f(f-1)/2 + (F-l)(F-l+1)/2
    => sum d_i^2 = 2S - f(f-1) - (F-l)(F-l+1) + (l - f)
with f/l the first/last spike positions (1-based) of the row.

Per chunk, engines:
  DMA : chunk load (~2.9us for 2000 cols; the DMA totals ~23us/core)
  ACT : y = 1-x cast f32->i16, accum -> W_c - k_c
  DVE : a = tensor_tensor_scan(op0=mult, op1=add, data0=data1=y,
            initial=prev_a[:, -1:])   (chained ages across chunks;
            2 cyc/elem, the critical engine: ~34us/core)
  ACT : copy a with accum -> S_c   (ACT has slack under the scans)
Once per core:
  DVE chunk0: stt(a==iota) accum -> f-1  (ages equal position iff before
              the first spike; chunk0 kept small)
  DVE last:   copy a[:, -1] -> F - l
First/last chunks are small to shorten pipeline fill/drain.
Host: merge (k, S, f, l) per half-row -> per-neuron CV -> loss.
"""

import numpy as np

B, T, N = 16, 2000, 512
L = B * T
NCORES = 8
NPC = N // NCORES
HALVES = 2
P = NPC * HALVES
F = L // HALVES
CHUNKS = (1000, 2000, 2000, 2000, 2000, 2000, 2000, 2000, 1000)
assert sum(CHUNKS) == F
NCH = len(CHUNKS)
# acc columns: [0:NCH] = W_c - k_c ; [NCH:2*NCH] = S_c ; [2*NCH] = f - 1 ;
# [2*NCH+1] = F - l
NACC = 2 * NCH + 2

_BUILD_CACHE = {}


def build_bass(P_=P):
    import concourse.bass as bass
    from concourse import bacc
    import concourse.mybir as mybir
    from concourse import tile

    nch = NCH
    W0 = CHUNKS[0]
    Wmax = max(CHUNKS)
    Alu = mybir.AluOpType
    AF = mybir.ActivationFunctionType
    f32 = mybir.dt.float32
    i16 = mybir.dt.int16

    nc = bacc.Bacc(trn_type="TRN2")
    x = nc.dram_tensor("x", (P_, F), f32, kind="ExternalInput")
    acc = nc.dram_tensor("acc", (P_, NACC), f32, kind="ExternalOutput")

    offs = []
    o = 0
    for w in CHUNKS:
        offs.append(o)
        o += w

    with tile.TileContext(nc) as tc:
        with tc.tile_pool(name="persist", bufs=1) as pp, \
             tc.tile_pool(name="work", bufs=3) as wp:
            iota = pp.tile([P_, W0], i16)
            nc.gpsimd.iota(iota[:], pattern=[[1, W0]], base=1,
                           channel_multiplier=0)
            accs = pp.tile([P_, NACC], f32)

            def load_and_cast(c):
                w = CHUNKS[c]
                lo = offs[c]
                xc = wp.tile([P_, Wmax], f32, tag="xc", name=f"xc{c}")
                nc.sync.dma_start(out=xc[:, :w], in_=x[:, lo:lo + w])
                y = wp.tile([P_, Wmax], i16, tag="y", name=f"y{c}")
                nc.scalar.activation(
                    out=y[:, :w], in_=xc[:, :w], func=AF.Copy,
                    scale=-1.0, bias=1.0,
                    accum_out=accs[:, c:c + 1])
                return y

            a_tiles = []

            def scan(c, y):
                w = CHUNKS[c]
                tag = "a0" if c == 0 else "a"
                a = wp.tile([P_, Wmax], i16, tag=tag, name=f"a{c}")
                init = (0.0 if c == 0
                        else a_tiles[c - 1][:, CHUNKS[c - 1] - 1:CHUNKS[c - 1]])
                nc.vector.tensor_tensor_scan(
                    out=a[:, :w], data0=y[:, :w], data1=y[:, :w],
                    initial=init, op0=Alu.mult, op1=Alu.add)
                a_tiles.append(a)

            def suma(c):
                w = CHUNKS[c]
                scr = wp.tile([P_, Wmax], f32, tag="scr", name=f"scr{c}")
                nc.scalar.activation(
                    out=scr[:, :w], in_=a_tiles[c][:, :w], func=AF.Copy,
                    accum_out=accs[:, nch + c:nch + c + 1])

            y_pend = load_and_cast(0)
            for c in range(nch):
                y_next = load_and_cast(c + 1) if c + 1 < nch else None
                scan(c, y_pend)
                if c >= 1:
                    suma(c - 1)
                if c == nch - 2:
                    # bulk of accs is final once suma(nch-3) retired: ship it
                    nc.sync.dma_start(out=acc[:, :2 * nch - 2],
                                      in_=accs[:, :2 * nch - 2])
                y_pend = y_next
            suma(nch - 1)
            # f - 1 = #(a_j == j) in chunk 0 (ages equal position exactly
            # until the first spike)
            eqf = wp.tile([P_, W0], i16, tag="eqf", name="eqf")
            nc.vector.scalar_tensor_tensor(
                out=eqf[:], in0=a_tiles[0][:, :W0], scalar=1.0, in1=iota[:],
                op0=Alu.mult, op1=Alu.is_equal,
                accum_out=accs[:, 2 * nch:2 * nch + 1])
            # F - l = final age
            wl = CHUNKS[-1]
            nc.vector.tensor_scalar(
                out=accs[:, 2 * nch + 1:2 * nch + 2],
                in0=a_tiles[-1][:, wl - 1:wl], scalar1=0.0, scalar2=None,
                op0=Alu.add)

            nc.sync.dma_start(out=acc[:, 2 * nch - 2:],
                              in_=accs[:, 2 * nch - 2:])
    nc.finalize()
    return nc


def get_bass():
    key = (F, CHUNKS, P)
    if key not in _BUILD_CACHE:
        _BUILD_CACHE[key] = build_bass()
    return _BUILD_CACHE[key]


def shard_input(output_spikes):
    x = np.asarray(output_spikes, dtype=np.float32)
    maps = []
    for c in range(NCORES):
        xc = x[:, :, c * NPC:(c + 1) * NPC]
        xt = np.ascontiguousarray(np.transpose(xc, (2, 0, 1))).reshape(NPC, L)
        maps.append({"x": xt.reshape(P, F)})
    return maps


def finish_host(acc_list, target_cv, in_maps=None, F_=F, nch=NCH):
    """Merge per-half-row (k, S, f, l) into the scalar loss (float64)."""
    target = np.asarray(target_cv, dtype=np.float64)
    widths = np.asarray(CHUNKS, dtype=np.float64)
    sq_sum = 0.0
    n_valid = 0
    for ci, acc in enumerate(acc_list):
        a = np.asarray(acc, dtype=np.float64)
        P_ = a.shape[0]
        k_h = np.rint((widths[None, :] - a[:, 0:nch]).sum(axis=1))
        k_c0 = np.rint(widths[0] - a[:, 0])
        S_h = a[:, nch:2 * nch].sum(axis=1)
        f_h = np.rint(a[:, 2 * nch] + 1.0)
        l_h = np.rint(F_ - a[:, 2 * nch + 1])
        n_neu = P_ // 2
        for n in range(n_neu):
            p1, p2 = 2 * n, 2 * n + 1
            stats = []
            for p in (p1, p2):
                kk = k_h[p]
                if kk < 1:
                    continue
                ff = f_h[p]
                if k_c0[p] < 1:
                    # first spike not in chunk 0: recover from host copy
                    row = in_maps[ci]["x"][p]
                    ff = float(np.argmax(row > 0) + 1)
                ll = l_h[p]
                s2 = (2.0 * S_h[p] - ff * (ff - 1.0)
                      - (F_ - ll) * (F_ - ll + 1.0) + (ll - ff))
                stats.append((kk, ff, ll, s2, p))
            if not stats:
                continue
            kt = sum(s[0] for s in stats)
            if kt < 3:
                continue
            if len(stats) == 2:
                (k1, f1, l1, s2a, _), (k2, f2, l2, s2b, _) = stats
                d_b = (F_ + f2) - l1
                s2 = s2a + s2b + d_b * d_b
                gf, gl = f1, F_ + l2
            else:
                kk, ff, ll, s2, p = stats[0]
                off = F_ if p == p2 else 0.0
                gf, gl = off + ff, off + ll
            s1 = gl - gf
            mean = s1 / (kt - 1.0)
            var = (s2 - s1 * s1 / (kt - 1.0)) / (kt - 2.0)
            std = np.sqrt(var) if var > 0 else 0.0
            if mean <= 0:
                continue
            cv = std / max(mean, 1e-12)
            d = cv - target[ci * NPC + n]
            sq_sum += d * d
            n_valid += 1
    return np.float32(sq_sum / max(n_valid, 1))


def ensure_ntff_hook(so_path="/opt/axon/libaxon_pjrt.so"):
    """Shim antenv.axon_hooks (absent in this image) so trace=True works.

    Mirrors trn_boot._ntff_profile_via_ctypes: drives NRT profiling via the
    axon PJRT .so's C ABI. Safe no-op if anything is missing.
    """
    import sys
    try:
        import antenv.axon_hooks  # noqa: F401
        return
    except ImportError:
        pass
    try:
        import ctypes
        import contextlib
        import types
        import os

        if not os.path.exists(so_path):
            return
        lib = ctypes.CDLL(so_path)
        if not hasattr(lib, "axon_start_nrt_profile"):
            return
        lib.axon_start_nrt_profile.argtypes = [
            ctypes.POINTER(ctypes.c_int64), ctypes.c_size_t]
        lib.axon_start_nrt_profile.restype = ctypes.c_int64
        lib.axon_stop_nrt_profile.argtypes = [ctypes.c_char_p]
        lib.axon_stop_nrt_profile.restype = ctypes.c_int64

        @contextlib.contextmanager
        def _hook(output_dir, device_ids):
            import jax
            jax.devices()
            if device_ids:
                ids = (ctypes.c_int64 * len(device_ids))(*device_ids)
                rc = lib.axon_start_nrt_profile(ids, len(device_ids))
            else:
                rc = lib.axon_start_nrt_profile(None, 0)
            if rc != 0:
                raise RuntimeError(f"axon_start_nrt_profile rc={rc}")
            try:
                yield
            finally:
                n = lib.axon_stop_nrt_profile(str(output_dir).encode())
                print(f"profile: {n} file(s) written to {output_dir}",
                      file=sys.stderr)

        mod = types.ModuleType("antenv.axon_hooks")
        mod.get_axon_ntff_profile_hook = lambda: _hook
        mod.set_axon_ntff_profile_hook = lambda h: None
        import antenv
        sys.modules["antenv.axon_hooks"] = mod
        antenv.axon_hooks = mod
    except Exception:
        pass


def kernel(output_spikes, target_cv):
    from concourse.bass_utils import run_bass_kernel_spmd

    ensure_ntff_hook()
    nc = get_bass()
    in_maps = shard_input(output_spikes)
    res = run_bass_kernel_spmd(nc, in_maps, core_ids=list(range(NCORES)))
    acc_list = [res.results[c]["acc"] for c in range(NCORES)]
    return finish_host(acc_list, target_cv, in_maps=in_maps)
